# revision 22
# baseline (speedup 1.0000x reference)
"""DeepseekV4-style MQA attention kernel for 8 Trainium2 NeuronCores.

Sharding:
- Shared projections (q_a/RMSNorm, q_b for ALL heads, k, v) are token-parallel:
  core c computes its 512-token chunk locally.
- k/v (single MQA head) are AllGathered (one fused collective, flat-packed).
- q is exchanged with one AllToAll: core c sends q[heads of core d, tokens of c]
  to d, receiving its 2 heads over all T. All communication overlaps the local
  projection compute; no gather feeds another projection.
- Attention + o_proj are head-parallel (2 heads per core, full T); each core
  emits a partial o_proj (row-slice of w_o) in bf16, host-summed in f32.

On-chip layout is feature-major: activations live as [features, tokens] so
every matmul contracts over the SBUF partition dim. All heavy matmuls are
bf16; PSUM accumulation is fp32. rel-err vs the fp32 reference ~5e-3.

Softmax uses no max-subtraction (scores are O(+-3)); the denominator comes
from an all-ones column appended to v. Causal masking is block-trimmed: the
4 diagonal 128-key tiles of each 512-query chunk compute only the surviving
query columns, masked by one triangular bf16 mask.

SBUF packing: 64-row tensors pack in pairs into 128-row tiles:
  Q2     = [q_rope_h0 (rows 0:64); q_rope_h1 (rows 64:128)]
  K2     = [k_rope; duplicate k_rope]
  trigQL = [cos; cos], trigSL = [sin; sin]  (one DVE op ropes both heads)
q feature order (24 tiles of 128): for core d: [nope(2d) | rope-pair(2d,2d+1)
| nope(2d+1)] — AllToAll chunk d is exactly core d's three attention tiles.
"""

import os
import numpy as np

B, S, HID = 2, 2048, 2048
H, DH, DR, DN = 16, 192, 64, 128
QL = 512
NCORES = 8
HPC = H // NCORES          # heads per core
T = B * S                  # global tokens
CH = 512                   # token chunk (= per-core local chunk)
NCH = T // CH
TPB = S // 128             # sk tiles per batch
KHID = HID // 128          # k-subtiles over HID
SCALE = DH ** -0.5
EPS = 1e-6
ROPE_THETA = 10000.0

KKV_N = 192 * CH + CH * 200          # flat fused k/v bounce elements
QIN_N = 24 * 128 * CH                # flat q alltoall elements
QCHUNK = 3 * 128 * CH                # per-destination q chunk

_CACHE = {}
LAST_RESULT = None


def _build_program():
    import concourse.tile as tile
    from concourse import bacc, mybir
    from concourse.masks import make_identity

    F32 = mybir.dt.float32
    F32R = mybir.dt.float32r
    BF16 = mybir.dt.bfloat16
    AF = mybir.ActivationFunctionType
    ALU = mybir.AluOpType

    nc = bacc.Bacc("TRN2", target_bir_lowering=False, debug=False)

    xc_d = nc.dram_tensor("xc", [HID, CH], BF16, kind="ExternalInput")
    wqa_d = nc.dram_tensor("wqa", [HID, QL], BF16, kind="ExternalInput")
    # full q_b, columns in 24-tile attention order (see module docstring)
    wqb_d = nc.dram_tensor("wqb", [QL, H * DH], BF16, kind="ExternalInput")
    # wkv columns: [k_nope(128) | v(192) | k_rope(64)]
    wkv_d = nc.dram_tensor("wkv", [HID, 384], BF16, kind="ExternalInput")
    wo_d = nc.dram_tensor("wo", [HPC * DH, HID], BF16, kind="ExternalInput")
    cosl_d = nc.dram_tensor("cosL", [DR, CH], BF16, kind="ExternalInput")
    sinl_d = nc.dram_tensor("sinL", [DR, CH], BF16, kind="ExternalInput")
    sink_d = nc.dram_tensor("sink", [1, HPC], F32, kind="ExternalInput")
    out_d = nc.dram_tensor("out", [T, HID], BF16, kind="ExternalOutput")

    with tile.TileContext(nc) as tc:
        with (
            tc.tile_pool(name="res", bufs=1) as res,
            tc.tile_pool(name="dram", bufs=1, space="DRAM") as dram,
        ):
            qa0 = res.tile([128, T], BF16)      # q^T nope head0
            Q2 = res.tile([128, T], BF16)       # q^T rope: h0 rows 0:64, h1 64:128
            qa1 = res.tile([128, T], BF16)      # q^T nope head1
            kT_a = res.tile([128, T], BF16)     # k^T nope (gathered)
            K2 = res.tile([128, T], BF16)       # k_rope rows 0:64, dup rows 64:128
            v_aug = res.tile([128, T // 128, 200], BF16)  # v natural + ones col
            sink_sb = res.tile([1, HPC], F32)
            ones_col = res.tile([128, 1], BF16)
            ones_row = res.tile([1, 128], F32R)
            ident = res.tile([128, 128], F32)
            identb = res.tile([128, 128], BF16)
            eps_sb = res.tile([1, 1], F32)
            trigQL = res.tile([128, CH], BF16)  # [cos; cos] local chunk
            trigSL = res.tile([128, CH], BF16)  # [sin; sin] local chunk
            dmask = res.tile([128, CH], BF16)   # triangle keep f>=p

            nc.sync.dma_start(sink_sb[:], sink_d[:])
            nc.sync.dma_start(trigQL[0:64, :], cosl_d[:])
            nc.sync.dma_start(trigQL[64:128, :], cosl_d[:])
            nc.sync.dma_start(trigSL[0:64, :], sinl_d[:])
            nc.sync.dma_start(trigSL[64:128, :], sinl_d[:])
            nc.gpsimd.memset(eps_sb[:], EPS)
            ones_cf = res.tile([128, 1], F32)
            nc.gpsimd.memset(ones_cf[:], 1.0)
            nc.vector.tensor_copy(out=ones_col[:], in_=ones_cf[:])
            ones_rf = res.tile([1, 128], F32)
            nc.gpsimd.memset(ones_rf[:], 1.0)
            nc.vector.tensor_copy(out=ones_row[:], in_=ones_rf[:])
            make_identity(nc, ident[:])
            nc.vector.tensor_copy(out=identb[:], in_=ident[:])
            dmask_f = res.tile([128, CH], F32)
            nc.gpsimd.memset(dmask_f[:], 1.0)
            nc.gpsimd.affine_select(
                out=dmask_f[:], in_=dmask_f[:],
                compare_op=ALU.is_ge, fill=0.0,
                base=0, channel_multiplier=-1, pattern=[[1, CH]],
            )
            nc.vector.tensor_copy(out=dmask[:], in_=dmask_f[:])

            # collective bounce buffers (flat DRAM)
            kkv_in = dram.tile([KKV_N], BF16)
            kkv_out = dram.tile([NCORES * KKV_N], BF16, addr_space="Shared")
            q_in = dram.tile([QIN_N], BF16)
            q_out = dram.tile([QIN_N], BF16)   # AllToAll: same size, Local

            # ---------------- phase 0: local projections + collectives ------
            with (
                tc.tile_pool(name="p0w", bufs=1) as p0w,
                tc.tile_pool(name="p0t", bufs=1) as p0t,
            ):
                wkv = p0w.tile([128, KHID, 384], BF16)
                nc.sync.dma_start(wkv[:], wkv_d[:].rearrange("(ko p) q -> p ko q", p=128))
                wqa = p0w.tile([128, KHID, QL], BF16)
                nc.sync.dma_start(wqa[:], wqa_d[:].rearrange("(ko p) q -> p ko q", p=128))
                wqb = p0w.tile([128, QL // 128, H * DH], BF16)
                nc.sync.dma_start(wqb[:], wqb_d[:].rearrange("(ko p) q -> p ko q", p=128))
                xts = []
                for k in range(KHID):
                    xt = p0w.tile([128, CH], BF16, name=f"xt{k}")
                    nc.sync.dma_start(xt[:], xc_d[k * 128:(k + 1) * 128, :])
                    xts.append(xt)

                # --- k/v first so the fused gather launches early ---
                with tc.tile_pool(name="p0kv_ps", bufs=1, space="PSUM") as kvps:
                    pkv = []
                    for m in range(3):
                        ps_kv = kvps.tile([128, CH], F32, tag=f"kv{m}", bufs=1,
                                          name=f"ps_kv{m}")
                        for k in range(KHID):
                            nc.tensor.matmul(
                                ps_kv[:], wkv[:, k, m * 128:(m + 1) * 128], xts[k][:],
                                start=(k == 0), stop=(k == KHID - 1),
                            )
                        pkv.append(ps_kv)
                    kT_loc = p0t.tile([128, CH], BF16, name="kT_loc")
                    nc.scalar.copy(out=kT_loc[:], in_=pkv[0][:])
                    vsa = p0t.tile([128, CH], BF16, name="vsa")
                    nc.scalar.copy(out=vsa[:], in_=pkv[1][:])
                    vsb = p0t.tile([64, CH], BF16, name="vsb")
                    nc.scalar.copy(out=vsb[:], in_=pkv[2][0:64, :])
                    # k rope: rows 64:128 of pkv[2]
                    k2_loc = p0t.tile([64, CH], BF16, name="k2_loc")
                    ktm = p0t.tile([64, CH], F32, name="ktm")
                    nc.vector.scalar_tensor_tensor(
                        out=ktm[0:32, :], in0=pkv[2][96:128, :], scalar=-1.0,
                        in1=trigSL[0:32, :], op0=ALU.mult, op1=ALU.mult)
                    nc.vector.scalar_tensor_tensor(
                        out=ktm[32:64, :], in0=pkv[2][64:96, :], scalar=1.0,
                        in1=trigSL[32:64, :], op0=ALU.mult, op1=ALU.mult)
                    ktc = p0t.tile([64, CH], F32, name="ktc")
                    nc.vector.tensor_mul(out=ktc[:], in0=pkv[2][64:128, :],
                                         in1=trigQL[0:64, :])
                    nc.vector.tensor_add(out=k2_loc[:], in0=ktc[:], in1=ktm[:])
                    # v natural layout (+ones col)
                    v_loc = p0t.tile([128, CH // 128, 200], BF16, name="v_loc")
                    ones_vc = p0t.tile([128, CH // 128, 8], F32, name="ones_vc")
                    nc.gpsimd.memset(ones_vc[:], 1.0)
                    nc.vector.tensor_copy(out=v_loc[:, :, 192:200], in_=ones_vc[:])
                    for blk in range(CH // 128):
                        bs = slice(blk * 128, (blk + 1) * 128)
                        pta = kvps.tile([128, 128], BF16, tag="tra", bufs=1, name="pta")
                        nc.tensor.transpose(pta[:], vsa[:, bs], identb[:])
                        nc.vector.tensor_copy(out=v_loc[:, blk, 0:128], in_=pta[:])
                        ptb = kvps.tile([128, 64], BF16, tag="trb", bufs=1, name="ptb")
                        nc.tensor.transpose(ptb[:], vsb[:, bs], identb[0:64, 0:64])
                        nc.vector.tensor_copy(out=v_loc[:, blk, 128:192], in_=ptb[:])

                KN = 128 * CH                   # kT elems
                K2N = 64 * CH                   # k_rope elems
                nc.sync.dma_start(
                    kkv_in[0:KN].rearrange("(p t) -> p t", p=128), kT_loc[:])
                nc.sync.dma_start(
                    kkv_in[KN:KN + K2N].rearrange("(p t) -> p t", p=64), k2_loc[:])
                nc.sync.dma_start(
                    kkv_in[KN + K2N:KKV_N].rearrange(
                        "(ti p f) -> p ti f", ti=CH // 128, p=128, f=200), v_loc[:])
                nc.gpsimd.collective_compute(
                    "AllGather", mybir.AluOpType.bypass,
                    replica_groups=[list(range(NCORES))],
                    ins=[kkv_in[:]], outs=[kkv_out[:]],
                )

                # --- q_a -> RMS -> normalized aT (local chunk) ---
                with tc.tile_pool(name="p0qa_ps", bufs=1, space="PSUM") as qaps:
                    ps_as = []
                    ps_ssq = qaps.tile([1, CH], F32, tag="ssq", bufs=1)
                    for m in range(4):
                        ps_a = qaps.tile([128, CH], F32, tag=f"a{m}", bufs=1,
                                         name=f"ps_a{m}")
                        for k in range(KHID):
                            nc.tensor.matmul(
                                ps_a[:], wqa[:, k, m * 128:(m + 1) * 128], xts[k][:],
                                start=(k == 0), stop=(k == KHID - 1),
                            )
                        ps_as.append(ps_a)
                        asq = p0t.tile([128, CH], BF16, name=f"asq{m}")
                        nc.scalar.activation(asq[:], ps_a[:], AF.Square)
                        nc.tensor.matmul(ps_ssq[:], ones_col[:], asq[:],
                                         start=(m == 0), stop=(m == 3))
                    sqr = p0t.tile([1, CH], F32, name="sqr")
                    nc.scalar.activation(sqr[:], ps_ssq[:], AF.Sqrt,
                                         bias=eps_sb[0:1, 0:1], scale=1.0 / QL)
                    rrow = p0t.tile([1, CH], F32R, name="rrow")
                    with nc.allow_low_precision(reason="f32r rsqrt feeds f32r matmul"):
                        nc.vector.reciprocal(out=rrow[:], in_=sqr[:])
                    ps_rb = qaps.tile([128, CH], F32, tag="rba", bufs=1, name="ps_rba")
                    nc.tensor.matmul(ps_rb[:], ones_row[:], rrow[:],
                                     start=True, stop=True)
                    rb_sb = p0t.tile([128, CH], F32, name="rb_sb")
                    nc.scalar.copy(out=rb_sb[:], in_=ps_rb[:])
                    aT_loc = p0t.tile([128, 4, CH], BF16, name="aT_loc")
                    for m in range(4):
                        nc.vector.tensor_mul(out=aT_loc[:, m, :], in0=ps_as[m][:],
                                             in1=rb_sb[:])

                # --- q_b for ALL heads on the local chunk, rope, AllToAll ---
                # rope tiles (m%3==1) first so their DVE tail is hidden and the
                # exchange can trigger right after the last (cheap) copy tile.
                q_loc = p0t.tile([128, 24, CH], BF16, name="q_loc")
                with tc.tile_pool(name="p0qb_ps", bufs=1, space="PSUM") as qbps:
                    for m in [3 * d + j for j in (1, 0, 2) for d in range(NCORES)]:
                        ps_q = qbps.tile([128, CH], F32, tag="q", bufs=3,
                                         name=f"ps_q{m}")
                        for k in range(QL // 128):
                            nc.tensor.matmul(
                                ps_q[:], wqb[:, k, m * 128:(m + 1) * 128],
                                aT_loc[:, k, :],
                                start=(k == 0), stop=(k == QL // 128 - 1),
                            )
                        if m % 3 != 1:
                            nc.scalar.copy(out=q_loc[:, m, :], in_=ps_q[:])
                        else:
                            # rope both heads at once (h_even 0:64, h_odd 64:128)
                            tms = p0t.tile([128, CH], F32, tag="tms", bufs=2,
                                           name="tms")
                            nc.vector.scalar_tensor_tensor(
                                out=tms[0:32, :], in0=ps_q[32:64, :], scalar=-1.0,
                                in1=trigSL[0:32, :], op0=ALU.mult, op1=ALU.mult)
                            nc.vector.scalar_tensor_tensor(
                                out=tms[32:64, :], in0=ps_q[0:32, :], scalar=1.0,
                                in1=trigSL[32:64, :], op0=ALU.mult, op1=ALU.mult)
                            nc.vector.scalar_tensor_tensor(
                                out=tms[64:96, :], in0=ps_q[96:128, :], scalar=-1.0,
                                in1=trigSL[64:96, :], op0=ALU.mult, op1=ALU.mult)
                            nc.vector.scalar_tensor_tensor(
                                out=tms[96:128, :], in0=ps_q[64:96, :], scalar=1.0,
                                in1=trigSL[96:128, :], op0=ALU.mult, op1=ALU.mult)
                            tmc = p0t.tile([128, CH], F32, tag="tmc", bufs=2,
                                           name="tmc")
                            nc.vector.tensor_mul(out=tmc[:], in0=ps_q[:],
                                                 in1=trigQL[:])
                            nc.vector.tensor_add(out=q_loc[:, m, :], in0=tmc[:],
                                                 in1=tms[:])
                nc.sync.dma_start(
                    q_in[:].rearrange("(m p t) -> p m t", m=24, p=128, t=CH),
                    q_loc[:])
                nc.gpsimd.collective_compute(
                    "AllToAll", mybir.AluOpType.bypass,
                    replica_groups=[list(range(NCORES))],
                    ins=[q_in[:]], outs=[q_out[:]],
                )

            # gather read-back: k^T / k_rope (dup) / v, then q (own heads)
            for s in range(NCORES):
                cs = slice(s * CH, (s + 1) * CH)
                kb = s * KKV_N
                KN = 128 * CH
                K2N = 64 * CH
                nc.sync.dma_start(
                    kT_a[:, cs],
                    kkv_out[kb:kb + KN].rearrange("(p t) -> p t", p=128))
                nc.sync.dma_start(
                    K2[0:64, cs],
                    kkv_out[kb + KN:kb + KN + K2N].rearrange("(p t) -> p t", p=64))
                nc.sync.dma_start(
                    K2[64:128, cs],
                    kkv_out[kb + KN:kb + KN + K2N].rearrange("(p t) -> p t", p=64))
                nc.sync.dma_start(
                    v_aug[:, s * 4:(s + 1) * 4, :],
                    kkv_out[kb + KN + K2N:kb + KKV_N].rearrange(
                        "(ti p f) -> p ti f", ti=CH // 128, p=128, f=200))
                qb_ = s * QCHUNK
                QN = 128 * CH
                nc.sync.dma_start(
                    qa0[:, cs],
                    q_out[qb_:qb_ + QN].rearrange("(p t) -> p t", p=128))
                nc.sync.dma_start(
                    Q2[:, cs],
                    q_out[qb_ + QN:qb_ + 2 * QN].rearrange("(p t) -> p t", p=128))
                nc.sync.dma_start(
                    qa1[:, cs],
                    q_out[qb_ + 2 * QN:qb_ + 3 * QN].rearrange("(p t) -> p t", p=128))

            # ---------------- phase 2: attention + o_proj ----------------
            with (
                tc.tile_pool(name="p2_w", bufs=1) as p2_w,
                tc.tile_pool(name="p2_e", bufs=4) as p2_e,
                tc.tile_pool(name="p2_s", bufs=2) as p2_s,
                tc.tile_pool(name="p2_o", bufs=3) as p2_o,
                tc.tile_pool(name="p2_ps", bufs=1, space="PSUM") as p2_ps,
            ):
                wo = p2_w.tile([128, 3, HID], BF16)
                nc.sync.dma_start(wo[:], wo_d[:].rearrange("(g p) n -> p g n", p=128))
                for b in range(B):
                    for sqc in range(S // CH):
                        sqbase = b * S + sqc * CH
                        nsk = (CH // 128) * (sqc + 1)
                        slabs = [
                            p2_s.tile([128, CH], BF16, tag=f"slab{g}", name=f"slab{g}")
                            for g in range(3)
                        ]
                        for h in range(HPC):
                            ps_o1 = p2_ps.tile([128, CH], F32, tag="o1", bufs=2,
                                               name="ps_o1")
                            ps_o2 = p2_ps.tile([65, CH], F32, tag="o2", bufs=2,
                                               name="ps_o2")
                            qn = qa0 if h == 0 else qa1
                            hb = 64 * h
                            for skt in range(nsk):
                                gt = b * TPB + skt
                                ks = slice(gt * 128, (gt + 1) * 128)
                                j = skt - (nsk - 4)   # diagonal index when >= 0
                                qoff = 128 * j if j > 0 else 0
                                w = CH - qoff
                                sq = slice(sqbase + qoff, sqbase + CH)
                                ps_s = p2_ps.tile([128, CH], F32, tag="s", bufs=2,
                                                  name="ps_s")
                                nc.tensor.matmul(ps_s[:, 0:w], kT_a[:, ks],
                                                 qn[:, sq], start=True, stop=False)
                                nc.tensor.matmul(ps_s[:, 0:w], K2[hb:hb + 64, ks],
                                                 Q2[hb:hb + 64, sq],
                                                 start=False, stop=True)
                                e = p2_e.tile([128, CH], BF16, tag="e", name="e")
                                nc.scalar.activation(e[:, 0:w], ps_s[:, 0:w],
                                                     AF.Exp, scale=SCALE)
                                if j >= 0:
                                    nc.vector.tensor_mul(out=e[:, 0:w], in0=e[:, 0:w],
                                                         in1=dmask[:, 0:w])
                                first, last = skt == 0, skt == nsk - 1
                                nc.tensor.matmul(ps_o1[:, qoff:], v_aug[:, gt, 0:128],
                                                 e[:, 0:w], start=first, stop=last)
                                nc.tensor.matmul(ps_o2[:, qoff:], v_aug[:, gt, 128:193],
                                                 e[:, 0:w], start=first, stop=last)
                            den = p2_e.tile([1, CH], F32, tag="den", name="den")
                            nc.scalar.activation(den[:], ps_o2[64:65, :], AF.Identity,
                                                 bias=sink_sb[0:1, h:h + 1])
                            rec = p2_e.tile([1, CH], F32R, tag="rec", name="rec")
                            with nc.allow_low_precision(reason="f32r recip for matmul"):
                                nc.vector.reciprocal(out=rec[:], in_=den[:])
                            ps_rb = p2_ps.tile([128, CH], F32, tag="s", bufs=2,
                                               name="ps_rb2")
                            nc.tensor.matmul(ps_rb[:], ones_row[:], rec[:],
                                             start=True, stop=True)
                            rb2 = p2_e.tile([128, CH], F32, tag="rb2", bufs=2,
                                            name="rb2")
                            nc.scalar.copy(out=rb2[:], in_=ps_rb[:])
                            if h == 0:
                                nc.vector.tensor_mul(out=slabs[0][:], in0=ps_o1[:],
                                                     in1=rb2[:])
                                nc.vector.tensor_mul(out=slabs[1][0:64, :],
                                                     in0=ps_o2[0:64, :],
                                                     in1=rb2[0:64, :])
                            else:
                                nc.vector.tensor_mul(out=slabs[1][64:128, :],
                                                     in0=ps_o1[0:64, :],
                                                     in1=rb2[0:64, :])
                                nc.vector.tensor_mul(out=slabs[2][0:64, :],
                                                     in0=ps_o1[64:128, :],
                                                     in1=rb2[64:128, :])
                                nc.vector.tensor_mul(out=slabs[2][64:128, :],
                                                     in0=ps_o2[0:64, :],
                                                     in1=rb2[0:64, :])
                        # o_proj partial for this (b, sqc)
                        for mt in range(CH // 128):
                            trow = sqbase + mt * 128
                            for nt in range(HID // 512):
                                ps_out = p2_ps.tile([128, 512], F32, tag="x", bufs=2,
                                                    name="ps_out")
                                for g in range(3):
                                    nc.tensor.matmul(
                                        ps_out[:], slabs[g][:, mt * 128:(mt + 1) * 128],
                                        wo[:, g, nt * 512:(nt + 1) * 512],
                                        start=(g == 0), stop=(g == 2),
                                    )
                                osb = p2_o.tile([128, 512], BF16, tag="osb", name="osb")
                                if (mt + nt) % 2 == 0:
                                    nc.vector.tensor_copy(out=osb[:], in_=ps_out[:])
                                else:
                                    nc.scalar.copy(out=osb[:], in_=ps_out[:])
                                nc.sync.dma_start(
                                    out_d[trow:trow + 128, nt * 512:(nt + 1) * 512],
                                    osb[:])

    nc.compile()
    return nc


def _make_runner(nc):
    """Mirror of bass2jax.run_bass_via_pjrt's multi-core path, but keeping the
    jitted callable so repeated executions don't re-trace/re-compile."""
    import jax
    import concourse.mybir as mybir
    from concourse import bass2jax
    from jax.experimental.shard_map import shard_map
    from jax.sharding import Mesh, PartitionSpec

    bass2jax.install_neuronx_cc_hook()

    partition_name = nc.partition_id_tensor.name if nc.partition_id_tensor else None
    in_names, out_names, out_avals = [], [], []
    for alloc in nc.m.functions[0].allocations:
        if not isinstance(alloc, mybir.MemoryLocationSet):
            continue
        name = alloc.memorylocations[0].name
        if alloc.kind == "ExternalInput":
            if name != partition_name:
                in_names.append(name)
        elif alloc.kind == "ExternalOutput":
            out_names.append(name)
            out_avals.append(jax.core.ShapedArray(
                tuple(alloc.tensor_shape), mybir.dt.np(alloc.dtype)))
    n_params = len(in_names)
    all_in_names = in_names + out_names
    if partition_name is not None:
        all_in_names.append(partition_name)
    donate = tuple(range(n_params, n_params + len(out_names)))

    def _body(*args):
        operands = list(args)
        if partition_name is not None:
            operands.append(bass2jax.partition_id_tensor())
        outs = bass2jax._bass_exec_p.bind(
            *operands,
            out_avals=tuple(out_avals),
            in_names=tuple(all_in_names),
            out_names=tuple(out_names),
            lowering_input_output_aliases=(),
            sim_require_finite=True,
            sim_require_nnan=True,
            nc=nc,
        )
        return tuple(outs)

    devices = jax.devices()[:NCORES]
    mesh = Mesh(np.asarray(devices), ("core",))
    n_all = n_params + len(out_names)
    sharded = jax.jit(
        shard_map(_body, mesh=mesh, in_specs=(PartitionSpec("core"),) * n_all,
                  out_specs=(PartitionSpec("core"),) * len(out_names), check_rep=False),
        donate_argnums=donate, keep_unused=True,
    )
    return {
        "fn": sharded, "in_names": in_names, "out_names": out_names,
        "out_avals": out_avals, "mesh": mesh,
    }


def _concat_inputs(runner, in_maps):
    return [
        np.concatenate([np.asarray(m[name]) for m in in_maps], axis=0)
        for name in runner["in_names"]
    ]


def _zero_outs(runner):
    return [
        np.zeros((NCORES * a.shape[0], *a.shape[1:]), a.dtype)
        for a in runner["out_avals"]
    ]


def run_on_device(runner, in_maps):
    out_arrs = runner["fn"](*_concat_inputs(runner, in_maps), *_zero_outs(runner))
    a = runner["out_avals"][0]
    return [
        np.asarray(out_arrs[0]).reshape(NCORES, *a.shape)[c]
        for c in range(NCORES)
    ]


def time_on_device(runner, in_maps, iters=30):
    """Slope timing: device-resident inputs, donation-chained outputs."""
    import jax
    import time as _time
    from jax.sharding import NamedSharding, PartitionSpec

    sh = NamedSharding(runner["mesh"], PartitionSpec("core"))
    dev_in = [jax.device_put(a, sh) for a in _concat_inputs(runner, in_maps)]
    outs = runner["fn"](*dev_in, *[jax.device_put(z, sh) for z in _zero_outs(runner)])
    outs = jax.block_until_ready(outs)

    def loop(n):
        nonlocal outs
        t0 = _time.perf_counter()
        for _ in range(n):
            outs = runner["fn"](*dev_in, *outs)
        jax.block_until_ready(outs)
        return _time.perf_counter() - t0

    n1 = max(2, iters // 3)
    t1 = loop(n1)
    t2 = loop(iters)
    per_iter = (t2 - t1) / (iters - n1) if t2 > t1 else t2 / iters
    return {"slope_s": per_iter, "t_small": t1 / n1, "t_big": t2 / iters}


def kernel(x, w_qa, q_norm_w, w_qb, w_k, w_v, w_o, attn_sink, position_ids):
    global LAST_RESULT
    import ml_dtypes
    BF = ml_dtypes.bfloat16

    x = np.asarray(x, dtype=np.float32)
    w_qa = np.ascontiguousarray(np.asarray(w_qa, dtype=np.float32))
    q_norm_w = np.asarray(q_norm_w, dtype=np.float32)
    w_qb = np.asarray(w_qb, dtype=np.float32)
    w_k = np.asarray(w_k, dtype=np.float32)
    w_v = np.asarray(w_v, dtype=np.float32)
    w_o = np.asarray(w_o, dtype=np.float32)
    attn_sink = np.asarray(attn_sink, dtype=np.float32)

    xT = np.ascontiguousarray(x.reshape(T, HID).T.astype(BF))
    wkv = np.ascontiguousarray(
        np.concatenate([w_k[:, :DN], w_v, w_k[:, DN:]], axis=1).astype(BF))
    wqb_eff = w_qb * q_norm_w[:, None]
    wqa_bf = np.ascontiguousarray(w_qa.astype(BF))

    # full q_b with columns permuted into the 24-tile attention order:
    # per core d: [nope(2d) | rope(2d) rope(2d+1) | nope(2d+1)]
    cols = []
    for d in range(NCORES):
        h0 = d * HPC
        qbs = wqb_eff[:, h0 * DH:(h0 + HPC) * DH]
        qb_h0, qb_h1 = qbs[:, :DH], qbs[:, DH:]
        cols += [qb_h0[:, :DN], qb_h0[:, DN:], qb_h1[:, DN:], qb_h1[:, :DN]]
    wqb_full = np.ascontiguousarray(np.concatenate(cols, axis=1).astype(BF))

    pos = np.asarray(position_ids).reshape(-1).astype(np.float32)
    inv = (1.0 / ROPE_THETA ** (np.arange(0, DR, 2, dtype=np.float32) / DR)).astype(np.float32)
    ang = pos[None, :] * inv[:, None]                     # [32, T]
    cosT = np.ascontiguousarray(
        np.concatenate([np.cos(ang), np.cos(ang)], 0).astype(BF))
    sinT = np.ascontiguousarray(
        np.concatenate([np.sin(ang), np.sin(ang)], 0).astype(BF))

    in_maps = []
    for c in range(NCORES):
        h0 = c * HPC
        wo_c = np.ascontiguousarray(w_o[h0 * DH:(h0 + HPC) * DH, :].astype(BF))
        sink_c = np.ascontiguousarray(
            np.exp(attn_sink[h0:h0 + HPC]).reshape(1, HPC).astype(np.float32))
        cl = slice(c * CH, (c + 1) * CH)
        in_maps.append({
            "xc": np.ascontiguousarray(xT[:, cl]),
            "wqa": wqa_bf, "wqb": wqb_full, "wkv": wkv, "wo": wo_c,
            "cosL": np.ascontiguousarray(cosT[:, cl]),
            "sinL": np.ascontiguousarray(sinT[:, cl]),
            "sink": sink_c,
        })

    if "runner" not in _CACHE:
        _CACHE["runner"] = _make_runner(_build_program())
    runner = _CACHE["runner"]
    LAST_RESULT = {"runner": runner, "in_maps": in_maps}

    outs = run_on_device(runner, in_maps)
    acc = outs[0].astype(np.float32)
    for c in range(1, NCORES):
        acc += outs[c].astype(np.float32)
    return acc.reshape(B, S, HID)


# revision 28
# speedup vs baseline: 1.3287x; 1.3287x over previous
"""DeepseekV4-style MQA attention kernel for 8 Trainium2 NeuronCores.

Sharding:
- Shared projections (q_a/RMSNorm, q_b for ALL heads, k, v) are token-parallel:
  core c computes its 512-token chunk locally.
- k/v (single MQA head) are AllGathered (one fused collective, flat-packed).
- q is exchanged with one AllToAll: core c sends q[heads of core d, tokens of c]
  to d, receiving its 2 heads over all T. All communication overlaps the local
  projection compute; no gather feeds another projection.
- Attention + o_proj are head-parallel (2 heads per core, full T); each core
  emits a partial o_proj (row-slice of w_o) in bf16, host-summed in f32.

On-chip layout is feature-major: activations live as [features, tokens] so
every matmul contracts over the SBUF partition dim. All heavy matmuls are
bf16; PSUM accumulation is fp32. rel-err vs the fp32 reference ~5e-3.

Softmax uses no max-subtraction (scores are O(+-3)); the denominator comes
from an all-ones column appended to v. Causal masking is block-trimmed: the
4 diagonal 128-key tiles of each 512-query chunk compute only the surviving
query columns, masked by one triangular bf16 mask.

SBUF packing: 64-row tensors pack in pairs into 128-row tiles:
  Q2     = [q_rope_h0 (rows 0:64); q_rope_h1 (rows 64:128)]
  K2     = [k_rope; duplicate k_rope]
  trigQL = [cos; cos], trigSL = [sin; sin]  (one DVE op ropes both heads)
q feature order (24 tiles of 128): for core d: [nope(2d) | rope-pair(2d,2d+1)
| nope(2d+1)] — AllToAll chunk d is exactly core d's three attention tiles.
"""

import os
import numpy as np

B, S, HID = 2, 2048, 2048
H, DH, DR, DN = 16, 192, 64, 128
QL = 512
NCORES = 8
HPC = H // NCORES          # heads per core
T = B * S                  # global tokens
CH = 512                   # token chunk (= per-core local chunk)
NCH = T // CH
TPB = S // 128             # sk tiles per batch
KHID = HID // 128          # k-subtiles over HID
SCALE = DH ** -0.5
EPS = 1e-6
ROPE_THETA = 10000.0

KKV_N = 192 * CH + CH * 200          # flat fused k/v bounce elements
QIN_N = 24 * 128 * CH                # flat q alltoall elements
QCHUNK = 3 * 128 * CH                # per-destination q chunk

_CACHE = {}
LAST_RESULT = None


def _build_program():
    import concourse.tile as tile
    from concourse import bacc, mybir
    from concourse.masks import make_identity

    F32 = mybir.dt.float32
    F32R = mybir.dt.float32r
    BF16 = mybir.dt.bfloat16
    AF = mybir.ActivationFunctionType
    ALU = mybir.AluOpType

    nc = bacc.Bacc("TRN2", target_bir_lowering=False, debug=False)

    xc_d = nc.dram_tensor("xc", [HID, CH], BF16, kind="ExternalInput")
    wqa_d = nc.dram_tensor("wqa", [HID, QL], BF16, kind="ExternalInput")
    # full q_b, columns in 24-tile attention order (see module docstring)
    wqb_d = nc.dram_tensor("wqb", [QL, H * DH], BF16, kind="ExternalInput")
    # wkv columns: [k_nope(128) | v(192) | k_rope(64)]
    wkv_d = nc.dram_tensor("wkv", [HID, 384], BF16, kind="ExternalInput")
    wo_d = nc.dram_tensor("wo", [HPC * DH, HID], BF16, kind="ExternalInput")
    cosl_d = nc.dram_tensor("cosL", [DR, CH], BF16, kind="ExternalInput")
    sinl_d = nc.dram_tensor("sinL", [DR, CH], BF16, kind="ExternalInput")
    sink_d = nc.dram_tensor("sink", [1, HPC], F32, kind="ExternalInput")
    out_d = nc.dram_tensor("out", [T, HID], BF16, kind="ExternalOutput")

    with tile.TileContext(nc) as tc:
        with (
            tc.tile_pool(name="res", bufs=1) as res,
            tc.tile_pool(name="dram", bufs=1, space="DRAM") as dram,
        ):
            qa0 = res.tile([128, T], BF16)      # q^T nope head0
            Q2 = res.tile([128, T], BF16)       # q^T rope: h0 rows 0:64, h1 64:128
            qa1 = res.tile([128, T], BF16)      # q^T nope head1
            kT_a = res.tile([128, T], BF16)     # k^T nope (gathered)
            K2 = res.tile([128, T], BF16)       # k_rope rows 0:64, dup rows 64:128
            v_aug = res.tile([128, T // 128, 200], BF16)  # v natural + ones col
            sink_sb = res.tile([1, HPC], F32)
            ones_col = res.tile([128, 1], BF16)
            ones_row = res.tile([1, 128], F32R)
            ident = res.tile([128, 128], F32)
            identb = res.tile([128, 128], BF16)
            eps_sb = res.tile([1, 1], F32)
            trigQL = res.tile([128, CH], BF16)  # [cos; cos] local chunk
            trigSL = res.tile([128, CH], BF16)  # [sin; sin] local chunk
            dmask = res.tile([128, CH], BF16)   # triangle keep f>=p

            nc.sync.dma_start(sink_sb[:], sink_d[:])
            nc.sync.dma_start(trigQL[0:64, :], cosl_d[:])
            nc.sync.dma_start(trigQL[64:128, :], cosl_d[:])
            nc.sync.dma_start(trigSL[0:64, :], sinl_d[:])
            nc.sync.dma_start(trigSL[64:128, :], sinl_d[:])
            nc.gpsimd.memset(eps_sb[:], EPS)
            ones_cf = res.tile([128, 1], F32)
            nc.gpsimd.memset(ones_cf[:], 1.0)
            nc.vector.tensor_copy(out=ones_col[:], in_=ones_cf[:])
            ones_rf = res.tile([1, 128], F32)
            nc.gpsimd.memset(ones_rf[:], 1.0)
            nc.vector.tensor_copy(out=ones_row[:], in_=ones_rf[:])
            make_identity(nc, ident[:])
            nc.vector.tensor_copy(out=identb[:], in_=ident[:])
            dmask_f = res.tile([128, CH], F32)
            nc.gpsimd.memset(dmask_f[:], 1.0)
            nc.gpsimd.affine_select(
                out=dmask_f[:], in_=dmask_f[:],
                compare_op=ALU.is_ge, fill=0.0,
                base=0, channel_multiplier=-1, pattern=[[1, CH]],
            )
            nc.vector.tensor_copy(out=dmask[:], in_=dmask_f[:])

            # collective bounce buffers (flat DRAM)
            kkv_in = dram.tile([KKV_N], BF16)
            kkv_out = dram.tile([NCORES * KKV_N], BF16, addr_space="Shared")
            q_in = dram.tile([QIN_N], BF16)
            q_out = dram.tile([QIN_N], BF16)   # AllToAll: same size, Local

            # ---------------- phase 0: local projections + collectives ------
            with (
                tc.tile_pool(name="p0w", bufs=1) as p0w,
                tc.tile_pool(name="p0t", bufs=1) as p0t,
            ):
                # interleave x-tile and per-ko weight loads so the first kv
                # matmul starts ~5us in, instead of waiting on monolithic
                # weight DMAs
                wkv = p0w.tile([128, KHID, 384], BF16)
                wqa = p0w.tile([128, KHID, QL], BF16)
                wqb = p0w.tile([128, QL // 128, H * DH], BF16)
                xts = []
                for k in range(KHID):
                    xt = p0w.tile([128, CH], BF16, name=f"xt{k}")
                    nc.sync.dma_start(xt[:], xc_d[k * 128:(k + 1) * 128, :])
                    nc.sync.dma_start(wkv[:, k, :], wkv_d[k * 128:(k + 1) * 128, :])
                    xts.append(xt)
                for k in range(KHID):
                    nc.sync.dma_start(wqa[:, k, :], wqa_d[k * 128:(k + 1) * 128, :])
                for k in range(QL // 128):
                    nc.sync.dma_start(wqb[:, k, :], wqb_d[k * 128:(k + 1) * 128, :])

                # --- k/v first so the fused gather launches early ---
                with tc.tile_pool(name="p0kv_ps", bufs=1, space="PSUM") as kvps:
                    pkv = []
                    for m in range(3):
                        ps_kv = kvps.tile([128, CH], F32, tag=f"kv{m}", bufs=1,
                                          name=f"ps_kv{m}")
                        for k in range(KHID):
                            nc.tensor.matmul(
                                ps_kv[:], wkv[:, k, m * 128:(m + 1) * 128], xts[k][:],
                                start=(k == 0), stop=(k == KHID - 1),
                            )
                        pkv.append(ps_kv)
                    kT_loc = p0t.tile([128, CH], BF16, name="kT_loc")
                    nc.scalar.copy(out=kT_loc[:], in_=pkv[0][:])
                    vsa = p0t.tile([128, CH], BF16, name="vsa")
                    nc.scalar.copy(out=vsa[:], in_=pkv[1][:])
                    vsb = p0t.tile([64, CH], BF16, name="vsb")
                    nc.scalar.copy(out=vsb[:], in_=pkv[2][0:64, :])
                    # k rope: rows 64:128 of pkv[2]
                    k2_loc = p0t.tile([64, CH], BF16, name="k2_loc")
                    ktm = p0t.tile([64, CH], F32, name="ktm")
                    nc.vector.scalar_tensor_tensor(
                        out=ktm[0:32, :], in0=pkv[2][96:128, :], scalar=-1.0,
                        in1=trigSL[0:32, :], op0=ALU.mult, op1=ALU.mult)
                    nc.vector.scalar_tensor_tensor(
                        out=ktm[32:64, :], in0=pkv[2][64:96, :], scalar=1.0,
                        in1=trigSL[32:64, :], op0=ALU.mult, op1=ALU.mult)
                    ktc = p0t.tile([64, CH], F32, name="ktc")
                    nc.vector.tensor_mul(out=ktc[:], in0=pkv[2][64:128, :],
                                         in1=trigQL[0:64, :])
                    nc.vector.tensor_add(out=k2_loc[:], in0=ktc[:], in1=ktm[:])
                    # v natural layout (+ones col)
                    v_loc = p0t.tile([128, CH // 128, 200], BF16, name="v_loc")
                    ones_vc = p0t.tile([128, CH // 128, 8], F32, name="ones_vc")
                    nc.gpsimd.memset(ones_vc[:], 1.0)
                    nc.vector.tensor_copy(out=v_loc[:, :, 192:200], in_=ones_vc[:])
                    for blk in range(CH // 128):
                        bs = slice(blk * 128, (blk + 1) * 128)
                        pta = kvps.tile([128, 128], BF16, tag="tra", bufs=1, name="pta")
                        nc.tensor.transpose(pta[:], vsa[:, bs], identb[:])
                        nc.vector.tensor_copy(out=v_loc[:, blk, 0:128], in_=pta[:])
                        ptb = kvps.tile([128, 64], BF16, tag="trb", bufs=1, name="ptb")
                        nc.tensor.transpose(ptb[:], vsb[:, bs], identb[0:64, 0:64])
                        nc.vector.tensor_copy(out=v_loc[:, blk, 128:192], in_=ptb[:])

                KN = 128 * CH                   # kT elems
                K2N = 64 * CH                   # k_rope elems
                nc.sync.dma_start(
                    kkv_in[0:KN].rearrange("(p t) -> p t", p=128), kT_loc[:])
                nc.sync.dma_start(
                    kkv_in[KN:KN + K2N].rearrange("(p t) -> p t", p=64), k2_loc[:])
                nc.sync.dma_start(
                    kkv_in[KN + K2N:KKV_N].rearrange(
                        "(ti p f) -> p ti f", ti=CH // 128, p=128, f=200), v_loc[:])
                nc.gpsimd.collective_compute(
                    "AllGather", mybir.AluOpType.bypass,
                    replica_groups=[list(range(NCORES))],
                    ins=[kkv_in[:]], outs=[kkv_out[:]],
                )

                # --- q_a -> RMS -> normalized aT (local chunk) ---
                with tc.tile_pool(name="p0qa_ps", bufs=1, space="PSUM") as qaps:
                    ps_as = []
                    ps_ssq = qaps.tile([1, CH], F32, tag="ssq", bufs=1)
                    for m in range(4):
                        ps_a = qaps.tile([128, CH], F32, tag=f"a{m}", bufs=1,
                                         name=f"ps_a{m}")
                        for k in range(KHID):
                            nc.tensor.matmul(
                                ps_a[:], wqa[:, k, m * 128:(m + 1) * 128], xts[k][:],
                                start=(k == 0), stop=(k == KHID - 1),
                            )
                        ps_as.append(ps_a)
                        asq = p0t.tile([128, CH], BF16, name=f"asq{m}")
                        nc.scalar.activation(asq[:], ps_a[:], AF.Square)
                        nc.tensor.matmul(ps_ssq[:], ones_col[:], asq[:],
                                         start=(m == 0), stop=(m == 3))
                    sqr = p0t.tile([1, CH], F32, name="sqr")
                    nc.scalar.activation(sqr[:], ps_ssq[:], AF.Sqrt,
                                         bias=eps_sb[0:1, 0:1], scale=1.0 / QL)
                    rrow_f = p0t.tile([1, CH], F32, name="rrow_f")
                    nc.vector.reciprocal_approx_fast(out=rrow_f[:], in_=sqr[:])
                    rrow = p0t.tile([1, CH], F32R, name="rrow")
                    with nc.allow_low_precision(reason="f32r feeds f32r matmul"):
                        nc.vector.tensor_copy(out=rrow[:], in_=rrow_f[:])
                    ps_rb = qaps.tile([128, CH], F32, tag="rba", bufs=1, name="ps_rba")
                    nc.tensor.matmul(ps_rb[:], ones_row[:], rrow[:],
                                     start=True, stop=True)
                    rb_sb = p0t.tile([128, CH], F32, name="rb_sb")
                    nc.scalar.copy(out=rb_sb[:], in_=ps_rb[:])
                    aT_loc = p0t.tile([128, 4, CH], BF16, name="aT_loc")
                    for m in range(4):
                        nc.vector.tensor_mul(out=aT_loc[:, m, :], in0=ps_as[m][:],
                                             in1=rb_sb[:])

                # --- q_b for ALL heads on the local chunk, rope, AllToAll ---
                # rope tiles (m%3==1) first so their DVE tail is hidden and the
                # exchange can trigger right after the last (cheap) copy tile.
                q_loc = p0t.tile([128, 24, CH], BF16, name="q_loc")
                with tc.tile_pool(name="p0qb_ps", bufs=1, space="PSUM") as qbps:
                    for m in [3 * d + j for j in (1, 0, 2) for d in range(NCORES)]:
                        ps_q = qbps.tile([128, CH], F32, tag="q", bufs=3,
                                         name=f"ps_q{m}")
                        for k in range(QL // 128):
                            nc.tensor.matmul(
                                ps_q[:], wqb[:, k, m * 128:(m + 1) * 128],
                                aT_loc[:, k, :],
                                start=(k == 0), stop=(k == QL // 128 - 1),
                            )
                        if m % 3 != 1:
                            nc.scalar.copy(out=q_loc[:, m, :], in_=ps_q[:])
                        else:
                            # rope both heads at once (h_even 0:64, h_odd 64:128)
                            tms = p0t.tile([128, CH], F32, tag="tms", bufs=2,
                                           name="tms")
                            nc.vector.scalar_tensor_tensor(
                                out=tms[0:32, :], in0=ps_q[32:64, :], scalar=-1.0,
                                in1=trigSL[0:32, :], op0=ALU.mult, op1=ALU.mult)
                            nc.vector.scalar_tensor_tensor(
                                out=tms[32:64, :], in0=ps_q[0:32, :], scalar=1.0,
                                in1=trigSL[32:64, :], op0=ALU.mult, op1=ALU.mult)
                            nc.vector.scalar_tensor_tensor(
                                out=tms[64:96, :], in0=ps_q[96:128, :], scalar=-1.0,
                                in1=trigSL[64:96, :], op0=ALU.mult, op1=ALU.mult)
                            nc.vector.scalar_tensor_tensor(
                                out=tms[96:128, :], in0=ps_q[64:96, :], scalar=1.0,
                                in1=trigSL[96:128, :], op0=ALU.mult, op1=ALU.mult)
                            tmc = p0t.tile([128, CH], F32, tag="tmc", bufs=2,
                                           name="tmc")
                            nc.vector.tensor_mul(out=tmc[:], in0=ps_q[:],
                                                 in1=trigQL[:])
                            nc.vector.tensor_add(out=q_loc[:, m, :], in0=tmc[:],
                                                 in1=tms[:])
                        # stream each finished tile to the bounce buffer so the
                        # AllToAll triggers right after the last tile
                        QN = 128 * CH
                        nc.sync.dma_start(
                            q_in[m * QN:(m + 1) * QN].rearrange("(p t) -> p t", p=128),
                            q_loc[:, m, :])
                nc.gpsimd.collective_compute(
                    "AllToAll", mybir.AluOpType.bypass,
                    replica_groups=[list(range(NCORES))],
                    ins=[q_in[:]], outs=[q_out[:]],
                )

            # gather read-back: k^T / k_rope (dup) / v, then q (own heads)
            for s in range(NCORES):
                cs = slice(s * CH, (s + 1) * CH)
                kb = s * KKV_N
                KN = 128 * CH
                K2N = 64 * CH
                nc.sync.dma_start(
                    kT_a[:, cs],
                    kkv_out[kb:kb + KN].rearrange("(p t) -> p t", p=128))
                nc.sync.dma_start(
                    K2[0:64, cs],
                    kkv_out[kb + KN:kb + KN + K2N].rearrange("(p t) -> p t", p=64))
                nc.sync.dma_start(
                    K2[64:128, cs],
                    kkv_out[kb + KN:kb + KN + K2N].rearrange("(p t) -> p t", p=64))
                nc.sync.dma_start(
                    v_aug[:, s * 4:(s + 1) * 4, :],
                    kkv_out[kb + KN + K2N:kb + KKV_N].rearrange(
                        "(ti p f) -> p ti f", ti=CH // 128, p=128, f=200))
                qb_ = s * QCHUNK
                QN = 128 * CH
                nc.sync.dma_start(
                    qa0[:, cs],
                    q_out[qb_:qb_ + QN].rearrange("(p t) -> p t", p=128))
                nc.sync.dma_start(
                    Q2[:, cs],
                    q_out[qb_ + QN:qb_ + 2 * QN].rearrange("(p t) -> p t", p=128))
                nc.sync.dma_start(
                    qa1[:, cs],
                    q_out[qb_ + 2 * QN:qb_ + 3 * QN].rearrange("(p t) -> p t", p=128))

            # ---------------- phase 2: attention + o_proj ----------------
            with (
                tc.tile_pool(name="p2_w", bufs=1) as p2_w,
                tc.tile_pool(name="p2_e", bufs=4) as p2_e,
                tc.tile_pool(name="p2_s", bufs=2) as p2_s,
                tc.tile_pool(name="p2_o", bufs=3) as p2_o,
                tc.tile_pool(name="p2_ps", bufs=1, space="PSUM") as p2_ps,
            ):
                wo = p2_w.tile([128, 3, HID], BF16)
                nc.sync.dma_start(wo[:], wo_d[:].rearrange("(g p) n -> p g n", p=128))
                for b in range(B):
                    for sqc in range(S // CH):
                        sqbase = b * S + sqc * CH
                        nsk = (CH // 128) * (sqc + 1)
                        slabs = [
                            p2_s.tile([128, CH], BF16, tag=f"slab{g}", name=f"slab{g}")
                            for g in range(3)
                        ]
                        # both heads interleaved: kT/v stationary weights are
                        # loaded once per key tile and serve both heads, and
                        # head h1's matmuls overlap head h0's exp/mask.
                        po1 = [p2_ps.tile([128, CH], F32, tag=f"o1h{h}", bufs=1,
                                          name=f"ps_o1h{h}") for h in range(HPC)]
                        po2 = [p2_ps.tile([65, CH], F32, tag=f"o2h{h}", bufs=1,
                                          name=f"ps_o2h{h}") for h in range(HPC)]
                        for skt in range(nsk):
                            gt = b * TPB + skt
                            ks = slice(gt * 128, (gt + 1) * 128)
                            j = skt - (nsk - 4)   # diagonal index when >= 0
                            qoff = 128 * j if j > 0 else 0
                            w = CH - qoff
                            sq = slice(sqbase + qoff, sqbase + CH)
                            pss = [p2_ps.tile([128, CH], F32, tag="s", bufs=2,
                                              name=f"ps_s{h}") for h in range(HPC)]
                            nc.tensor.matmul(pss[0][:, 0:w], kT_a[:, ks],
                                             qa0[:, sq], start=True, stop=False)
                            nc.tensor.matmul(pss[1][:, 0:w], kT_a[:, ks],
                                             qa1[:, sq], start=True, stop=False)
                            nc.tensor.matmul(pss[0][:, 0:w], K2[0:64, ks],
                                             Q2[0:64, sq], start=False, stop=True)
                            nc.tensor.matmul(pss[1][:, 0:w], K2[64:128, ks],
                                             Q2[64:128, sq], start=False, stop=True)
                            es = []
                            for h in range(HPC):
                                e = p2_e.tile([128, CH], BF16, tag=f"e{h}",
                                              name=f"e{h}")
                                nc.scalar.activation(e[:, 0:w], pss[h][:, 0:w],
                                                     AF.Exp, scale=SCALE)
                                if j >= 0:
                                    nc.vector.tensor_mul(out=e[:, 0:w], in0=e[:, 0:w],
                                                         in1=dmask[:, 0:w])
                                es.append(e)
                            first, last = skt == 0, skt == nsk - 1
                            nc.tensor.matmul(po1[0][:, qoff:], v_aug[:, gt, 0:128],
                                             es[0][:, 0:w], start=first, stop=last)
                            nc.tensor.matmul(po1[1][:, qoff:], v_aug[:, gt, 0:128],
                                             es[1][:, 0:w], start=first, stop=last)
                            nc.tensor.matmul(po2[0][:, qoff:], v_aug[:, gt, 128:193],
                                             es[0][:, 0:w], start=first, stop=last)
                            nc.tensor.matmul(po2[1][:, qoff:], v_aug[:, gt, 128:193],
                                             es[1][:, 0:w], start=first, stop=last)
                        for h in range(HPC):
                            den = p2_e.tile([1, CH], F32, tag="den", name="den")
                            nc.scalar.activation(den[:], po2[h][64:65, :], AF.Identity,
                                                 bias=sink_sb[0:1, h:h + 1])
                            rec_f = p2_e.tile([1, CH], F32, tag="rec_f", name="rec_f")
                            nc.vector.reciprocal_approx_fast(out=rec_f[:], in_=den[:])
                            rec = p2_e.tile([1, CH], F32R, tag="rec", name="rec")
                            with nc.allow_low_precision(reason="f32r recip for matmul"):
                                nc.vector.tensor_copy(out=rec[:], in_=rec_f[:])
                            ps_rb = p2_ps.tile([128, CH], F32, tag="x", bufs=2,
                                               name="ps_rb2")
                            nc.tensor.matmul(ps_rb[:], ones_row[:], rec[:],
                                             start=True, stop=True)
                            rb2 = p2_e.tile([128, CH], F32, tag="rb2", bufs=2,
                                            name="rb2")
                            nc.scalar.copy(out=rb2[:], in_=ps_rb[:])
                            if h == 0:
                                nc.vector.tensor_mul(out=slabs[0][:], in0=po1[0][:],
                                                     in1=rb2[:])
                                nc.vector.tensor_mul(out=slabs[1][0:64, :],
                                                     in0=po2[0][0:64, :],
                                                     in1=rb2[0:64, :])
                            else:
                                nc.vector.tensor_mul(out=slabs[1][64:128, :],
                                                     in0=po1[1][0:64, :],
                                                     in1=rb2[0:64, :])
                                nc.vector.tensor_mul(out=slabs[2][0:64, :],
                                                     in0=po1[1][64:128, :],
                                                     in1=rb2[64:128, :])
                                nc.vector.tensor_mul(out=slabs[2][64:128, :],
                                                     in0=po2[1][0:64, :],
                                                     in1=rb2[0:64, :])
                        # o_proj partial for this (b, sqc)
                        for mt in range(CH // 128):
                            trow = sqbase + mt * 128
                            for nt in range(HID // 512):
                                ps_out = p2_ps.tile([128, 512], F32, tag="x", bufs=2,
                                                    name="ps_out")
                                for g in range(3):
                                    nc.tensor.matmul(
                                        ps_out[:], slabs[g][:, mt * 128:(mt + 1) * 128],
                                        wo[:, g, nt * 512:(nt + 1) * 512],
                                        start=(g == 0), stop=(g == 2),
                                    )
                                osb = p2_o.tile([128, 512], BF16, tag="osb", name="osb")
                                if (mt + nt) % 2 == 0:
                                    nc.vector.tensor_copy(out=osb[:], in_=ps_out[:])
                                else:
                                    nc.scalar.copy(out=osb[:], in_=ps_out[:])
                                nc.sync.dma_start(
                                    out_d[trow:trow + 128, nt * 512:(nt + 1) * 512],
                                    osb[:])

    nc.compile()
    return nc


def _make_runner(nc):
    """Mirror of bass2jax.run_bass_via_pjrt's multi-core path, but keeping the
    jitted callable so repeated executions don't re-trace/re-compile."""
    import jax
    import concourse.mybir as mybir
    from concourse import bass2jax
    from jax.experimental.shard_map import shard_map
    from jax.sharding import Mesh, PartitionSpec

    bass2jax.install_neuronx_cc_hook()

    partition_name = nc.partition_id_tensor.name if nc.partition_id_tensor else None
    in_names, out_names, out_avals = [], [], []
    for alloc in nc.m.functions[0].allocations:
        if not isinstance(alloc, mybir.MemoryLocationSet):
            continue
        name = alloc.memorylocations[0].name
        if alloc.kind == "ExternalInput":
            if name != partition_name:
                in_names.append(name)
        elif alloc.kind == "ExternalOutput":
            out_names.append(name)
            out_avals.append(jax.core.ShapedArray(
                tuple(alloc.tensor_shape), mybir.dt.np(alloc.dtype)))
    n_params = len(in_names)
    all_in_names = in_names + out_names
    if partition_name is not None:
        all_in_names.append(partition_name)
    donate = tuple(range(n_params, n_params + len(out_names)))

    def _body(*args):
        operands = list(args)
        if partition_name is not None:
            operands.append(bass2jax.partition_id_tensor())
        outs = bass2jax._bass_exec_p.bind(
            *operands,
            out_avals=tuple(out_avals),
            in_names=tuple(all_in_names),
            out_names=tuple(out_names),
            lowering_input_output_aliases=(),
            sim_require_finite=True,
            sim_require_nnan=True,
            nc=nc,
        )
        return tuple(outs)

    devices = jax.devices()[:NCORES]
    mesh = Mesh(np.asarray(devices), ("core",))
    n_all = n_params + len(out_names)
    sharded = jax.jit(
        shard_map(_body, mesh=mesh, in_specs=(PartitionSpec("core"),) * n_all,
                  out_specs=(PartitionSpec("core"),) * len(out_names), check_rep=False),
        donate_argnums=donate, keep_unused=True,
    )
    return {
        "fn": sharded, "in_names": in_names, "out_names": out_names,
        "out_avals": out_avals, "mesh": mesh,
    }


def _concat_inputs(runner, in_maps):
    return [
        np.concatenate([np.asarray(m[name]) for m in in_maps], axis=0)
        for name in runner["in_names"]
    ]


def _zero_outs(runner):
    return [
        np.zeros((NCORES * a.shape[0], *a.shape[1:]), a.dtype)
        for a in runner["out_avals"]
    ]


def run_on_device(runner, in_maps):
    out_arrs = runner["fn"](*_concat_inputs(runner, in_maps), *_zero_outs(runner))
    a = runner["out_avals"][0]
    return [
        np.asarray(out_arrs[0]).reshape(NCORES, *a.shape)[c]
        for c in range(NCORES)
    ]


def time_on_device(runner, in_maps, iters=30):
    """Slope timing: device-resident inputs, donation-chained outputs."""
    import jax
    import time as _time
    from jax.sharding import NamedSharding, PartitionSpec

    sh = NamedSharding(runner["mesh"], PartitionSpec("core"))
    dev_in = [jax.device_put(a, sh) for a in _concat_inputs(runner, in_maps)]
    outs = runner["fn"](*dev_in, *[jax.device_put(z, sh) for z in _zero_outs(runner)])
    outs = jax.block_until_ready(outs)

    def loop(n):
        nonlocal outs
        t0 = _time.perf_counter()
        for _ in range(n):
            outs = runner["fn"](*dev_in, *outs)
        jax.block_until_ready(outs)
        return _time.perf_counter() - t0

    n1 = max(2, iters // 3)
    t1 = loop(n1)
    t2 = loop(iters)
    per_iter = (t2 - t1) / (iters - n1) if t2 > t1 else t2 / iters
    return {"slope_s": per_iter, "t_small": t1 / n1, "t_big": t2 / iters}


def kernel(x, w_qa, q_norm_w, w_qb, w_k, w_v, w_o, attn_sink, position_ids):
    global LAST_RESULT
    import ml_dtypes
    BF = ml_dtypes.bfloat16

    x = np.asarray(x, dtype=np.float32)
    w_qa = np.ascontiguousarray(np.asarray(w_qa, dtype=np.float32))
    q_norm_w = np.asarray(q_norm_w, dtype=np.float32)
    w_qb = np.asarray(w_qb, dtype=np.float32)
    w_k = np.asarray(w_k, dtype=np.float32)
    w_v = np.asarray(w_v, dtype=np.float32)
    w_o = np.asarray(w_o, dtype=np.float32)
    attn_sink = np.asarray(attn_sink, dtype=np.float32)

    xT = np.ascontiguousarray(x.reshape(T, HID).T.astype(BF))
    wkv = np.ascontiguousarray(
        np.concatenate([w_k[:, :DN], w_v, w_k[:, DN:]], axis=1).astype(BF))
    wqb_eff = w_qb * q_norm_w[:, None]
    wqa_bf = np.ascontiguousarray(w_qa.astype(BF))

    # full q_b with columns permuted into the 24-tile attention order:
    # per core d: [nope(2d) | rope(2d) rope(2d+1) | nope(2d+1)]
    cols = []
    for d in range(NCORES):
        h0 = d * HPC
        qbs = wqb_eff[:, h0 * DH:(h0 + HPC) * DH]
        qb_h0, qb_h1 = qbs[:, :DH], qbs[:, DH:]
        cols += [qb_h0[:, :DN], qb_h0[:, DN:], qb_h1[:, DN:], qb_h1[:, :DN]]
    wqb_full = np.ascontiguousarray(np.concatenate(cols, axis=1).astype(BF))

    pos = np.asarray(position_ids).reshape(-1).astype(np.float32)
    inv = (1.0 / ROPE_THETA ** (np.arange(0, DR, 2, dtype=np.float32) / DR)).astype(np.float32)
    ang = pos[None, :] * inv[:, None]                     # [32, T]
    cosT = np.ascontiguousarray(
        np.concatenate([np.cos(ang), np.cos(ang)], 0).astype(BF))
    sinT = np.ascontiguousarray(
        np.concatenate([np.sin(ang), np.sin(ang)], 0).astype(BF))

    in_maps = []
    for c in range(NCORES):
        h0 = c * HPC
        wo_c = np.ascontiguousarray(w_o[h0 * DH:(h0 + HPC) * DH, :].astype(BF))
        sink_c = np.ascontiguousarray(
            np.exp(attn_sink[h0:h0 + HPC]).reshape(1, HPC).astype(np.float32))
        cl = slice(c * CH, (c + 1) * CH)
        in_maps.append({
            "xc": np.ascontiguousarray(xT[:, cl]),
            "wqa": wqa_bf, "wqb": wqb_full, "wkv": wkv, "wo": wo_c,
            "cosL": np.ascontiguousarray(cosT[:, cl]),
            "sinL": np.ascontiguousarray(sinT[:, cl]),
            "sink": sink_c,
        })

    if "runner" not in _CACHE:
        _CACHE["runner"] = _make_runner(_build_program())
    runner = _CACHE["runner"]
    LAST_RESULT = {"runner": runner, "in_maps": in_maps}

    outs = run_on_device(runner, in_maps)
    acc = outs[0].astype(np.float32)
    for c in range(1, NCORES):
        acc += outs[c].astype(np.float32)
    return acc.reshape(B, S, HID)
